# revision 13
# baseline (speedup 1.0000x reference)
"""EnsembleGRU Trainium2 kernel, v11.

Math (per ensemble member e, H=1):
    gi = x @ Wc^T + bc   (Wc = Wih @ Wl folded, bc includes bih/bhh/bl terms)
    scan over W steps:
        r  = sigmoid(gi_r + a*h)            a = whh[0]
        z  = sigmoid(gi_z + b*h)            b = whh[1]
        n  = tanh(gi_n + r*(c*h + d))       c = whh[2], d = bhh[2]
        h' = (1-z)*n + z*h = q - u,  q = z*h, u = (z-1)*n

Sharding: E=16 members over 8 cores (2 per core), zero communication.
Lane layout per core: partition p = e_loc*64 + p', free col c in 0..39,
bi = p'*40 + c (5120 lanes = 128 x 40).

Schedule (v11): all scan tiles fp16; every chain op is a TensorTensor
(2x DVE mode, 81ns) -- TensorScalarPtr supports no fast mode, so
per-partition constants (a, b, c) are pre-broadcast to [128,CC] fp16
tensors in the prologue and stt survives only in off-chain ops:
  - chain: sig_r -> v = hc*r -> an = gi_n + v -> tanh -> u_a = zm1a*n
    -> ar = p1r - u_a -> sig_r(w+1).
  - off-chain, z-gated, fills the tanh window: zm1 = z-1,
    zm1a = zm1*a_t, q = z*h, p1r = a*q + gi_r' (stt), p1z (stt).
  - after ar: u = zm1*n (fills ar's sem-wait bubble), az = -b*u + p1z
    (stt), h' = q - u, hc = h'*c_t for the next step's v.
  - gcp (PSUM->SBUF gi prefetch, f32->f16) runs on the Pool engine --
    free there, and keeps the big PSUM-read off the DVE queue.
  - prologue: cst + x0 + dg DMAs issued in that order on the SP hwdge
    queue (x0 small because group 0 covers 1 step); matmul group sizes
    [1,7,8x7] so the scan starts ~端6us instead of ~18us.
"""

import numpy as np

W, E, B, I, F = 64, 16, 256, 10, 8
BI = B * I            # 2560
NCORES = 8
E_LOC = E // NCORES   # 2
PP = 64               # partitions per member
CC = BI // PP         # 40 free cols per step
G = 3                 # gates

WGS = [1, 1, 2, 4, 8, 12, 12, 12, 12]  # w-steps per matmul group (sum = 64)
NG = len(WGS)
GSTART = [sum(WGS[:k]) for k in range(NG)]
NDIAG = 27            # 24 (g,f) Wc diags + 3 bias diags

_CACHED = {}


def _build_nc(d_nonzero: bool, rep: int = 1):
    import contextlib

    import concourse.bacc as bacc
    import concourse.mybir as mybir
    from concourse.tile import TileContext

    AL = mybir.AluOpType
    AF = mybir.ActivationFunctionType
    f32 = mybir.dt.float32
    f16 = mybir.dt.float16

    nc = bacc.Bacc("TRN2", target_bir_lowering=False)

    xh = nc.dram_tensor("xh", [128, F, W, CC], f16, kind="ExternalInput")
    dg = nc.dram_tensor("dg", [128, NDIAG * 128], f16, kind="ExternalInput")
    cst = nc.dram_tensor("cst", [128, 7 + CC], f32, kind="ExternalInput")
    out = nc.dram_tensor("out", [128, W * CC], f16, kind="ExternalOutput")

    with TileContext(nc) as tc:
        with (
            tc.tile_pool(name="const", bufs=1) as constp,
            tc.tile_pool(name="xp", bufs=3) as xp,
            tc.tile_pool(name="gip", bufs=2, space="PSUM") as gip,
            tc.tile_pool(name="scan", bufs=3) as scanp,
            tc.tile_pool(name="outp", bufs=1) as outp,
        ):
            dg_sb = constp.tile([128, NDIAG * 128], f16, tag="dg")
            cst_sb = constp.tile([128, 7 + CC], f32, tag="cst")
            ones = constp.tile([128, 12 * CC], f16, tag="ones")
            bct = constp.tile([128, 3 * CC], f16, tag="bct")  # [a_t|c_t|d_t]
            actw = constp.tile([128, 8], f32, tag="actw")
            out_sb = outp.tile([128, (W + 1) * CC], f16, tag="out")

            x_tiles = {}
            x_t0 = xp.tile([128, F * WGS[0] * CC], f16, tag="x")

            # DMA order on the SP hwdge queue: dg first (the long pole
            # gating all matmuls), then cst and x0 (small).
            nc.sync.dma_start(dg_sb[:], dg[:])
            nc.sync.dma_start(cst_sb[:], cst[:])
            nc.sync.dma_start(
                x_t0[:].rearrange("p (f w c) -> p f w c", f=F, c=CC),
                xh[:, :, 0 : WGS[0], :],
            )
            x_tiles[0] = x_t0

            nc.vector.memset(ones[:], 1.0)
            # preload the activation LUT set off the critical path
            nc.scalar.activation(actw[:], ones[:, 0:8], AF.Sigmoid)
            nc.scalar.activation(actw[:], ones[:, 0:8], AF.Tanh)

            a_s = cst_sb[:, 0:1]
            b_s = cst_sb[:, 1:2]
            c_s = cst_sb[:, 2:3]
            d_s = cst_sb[:, 3:4]
            na_s = cst_sb[:, 5:6]  # -a
            nb_s = cst_sb[:, 6:7]  # -b

            # broadcast consts to fp16 tensors (TT chain operands)
            nc.vector.tensor_scalar(bct[:, 0:CC], ones[:, 0:CC], a_s, None, AL.mult)
            nc.vector.tensor_scalar(bct[:, CC : 2 * CC], ones[:, 0:CC], c_s, None, AL.mult)
            if d_nonzero:
                nc.vector.tensor_scalar(
                    bct[:, 2 * CC : 3 * CC], ones[:, 0:CC], d_s, None, AL.mult
                )
            # h0 into slot 0 (f32 -> f16 cast)
            nc.vector.tensor_copy(out_sb[:, 0:CC], cst_sb[:, 7 : 7 + CC])

            a_t = bct[:, 0:CC]
            c_t = bct[:, CC : 2 * CC]
            d_t = bct[:, 2 * CC : 3 * CC]

            loop_cm = tc.For_i(0, rep, 1) if rep > 1 else contextlib.nullcontext()
            with loop_cm:
                _body(
                    nc, tc, xp, gip, scanp, xh, out, dg_sb, ones, out_sb,
                    a_s, b_s, c_s, d_s, na_s, nb_s, a_t, c_t, d_t,
                    AL, AF, f32, f16, d_nonzero, x_tiles,
                )

    nc.finalize()
    return nc


def _body(
    nc, tc, xp, gip, scanp, xh, out, dg_sb, ones, out_sb,
    a_s, b_s, c_s, d_s, na_s, nb_s, a_t, c_t, d_t,
    AL, AF, f32, f16, d_nonzero, x_tiles,
):
    gi_tiles = {}

    def emit_x_load(k):
        w0, wg = GSTART[k], WGS[k]
        x_t = xp.tile([128, F * wg * CC], f16, tag="x")
        nc.sync.dma_start(
            x_t[:].rearrange("p (f w c) -> p f w c", f=F, c=CC),
            xh[:, :, w0 : w0 + wg, :],
        )
        x_tiles[k] = x_t

    def emit_mm_group(k):
        # PSUM ring (bufs=2): this alloc WARs all group k-2 readers emitted
        # so far, so pacing the emission inside the scan keeps order safe.
        wg = WGS[k]
        gi_ps = gip.tile([128, G * 512], f32, tag="gi")
        gi_tiles[k] = gi_ps
        x_t = x_tiles[k]
        for g in range(G):
            reg = gi_ps[:, g * 512 : g * 512 + wg * CC]
            # bias first (start=True clears the accumulation region)
            nc.tensor.matmul(
                reg,
                dg_sb[:, (24 + g) * 128 : (25 + g) * 128],
                ones[:, : wg * CC],
                start=True,
                stop=False,
                skip_group_check=True,
            )
            for f in range(F):
                nc.tensor.matmul(
                    reg,
                    dg_sb[:, (g * F + f) * 128 : (g * F + f + 1) * 128],
                    x_t[:, f * wg * CC : (f + 1) * wg * CC],
                    start=False,
                    stop=(f == F - 1),
                    skip_group_check=True,
                )

    emit_x_load(1)
    emit_x_load(2)
    emit_mm_group(0)
    emit_mm_group(1)

    def h_ap(w):
        return out_sb[:, w * CC : (w + 1) * CC]

    def emit_out_dma(k):
        w0, wg = GSTART[k], WGS[k]
        nc.sync.dma_start(
            out[:, w0 * CC : (w0 + wg) * CC],
            out_sb[:, (w0 + 1) * CC : (w0 + wg + 1) * CC],
        )

    gends = {GSTART[k] + WGS[k] - 1: k for k in range(NG - 1)}
    gfirst = {GSTART[k]: k for k in range(NG)}

    stt = nc.vector.scalar_tensor_tensor
    tt = nc.vector.tensor_tensor

    def emit_gcp(w):
        # whole-step gi prefetch PSUM -> SBUF on the POOL engine (one
        # strided copy, [r|z|n], f32 -> f16 cast): keeps the PSUM read off
        # the DVE queue entirely.
        k = next(i for i in range(NG) if GSTART[i] <= w < GSTART[i] + WGS[i])
        wl = w - GSTART[k]
        gcp = scanp.tile([128, G * CC], f16, tag="gcp")
        nc.vector.tensor_copy(
            gcp[:].rearrange("p (g c) -> p g c", g=G),
            gi_tiles[k][:]
            .rearrange("p (g x) -> p g x", g=G)[:, :, wl * CC : (wl + 1) * CC],
        )
        return gcp

    # step 0 prologue: ar/az/hc from h0
    gcp = emit_gcp(0)
    ar = scanp.tile([128, CC], f16, tag="ar")
    az = scanp.tile([128, CC], f16, tag="az")
    stt(ar[:], h_ap(0), a_s, gcp[:, 0:CC], AL.mult, AL.add)
    stt(az[:], h_ap(0), b_s, gcp[:, CC : 2 * CC], AL.mult, AL.add)
    r_t = scanp.tile([128, CC], f16, tag="r")
    z_t = scanp.tile([128, CC], f16, tag="z")
    nc.scalar.activation(r_t[:], ar[:], AF.Sigmoid)
    nc.scalar.activation(z_t[:], az[:], AF.Sigmoid)
    for w in range(W):
        h = h_ap(w)
        last = w == W - 1
        # paced matmul groups and x loads: at the first step of group j
        # (j >= 1), emit group j+1 (PSUM ring bufs=2: its buffer WARs
        # group j-1, fully consumed in program order) and load x for j+2.
        if w in gfirst and gfirst[w] >= 1:
            k2 = gfirst[w] + 1
            if k2 < NG:
                if k2 + 1 not in x_tiles and k2 + 1 < NG:
                    emit_x_load(k2 + 1)
                emit_mm_group(k2)

        gcp_n = None if last else emit_gcp(w + 1)
        v = scanp.tile([128, CC], f16, tag="v")
        an = scanp.tile([128, CC], f16, tag="an")
        q = scanp.tile([128, CC], f16, tag="q")
        n_t = scanp.tile([128, CC], f16, tag="n")
        u = scanp.tile([128, CC], f16, tag="u")

        # chain: v, an (DVE), tanh (ACT), u, ar (DVE), sigmoid (ACT); the
        # off-chain q/p1r/p1z sit after u in the queue and execute in the
        # tanh shadow via the wait-queue bypass (all SBUF-fast operands).
        stt(v[:], h, c_s, r_t[:, 0:CC], AL.mult, AL.mult)
        if d_nonzero:
            stt(v[:], r_t[:, 0:CC], d_s, v[:], AL.mult, AL.add)
        tt(an[:], gcp[:, 2 * CC :], v[:], AL.add)
        nc.scalar.activation(n_t[:], an[:], AF.Tanh)
        stt(u[:], z_t[:], 1.0, n_t[:], AL.subtract, AL.mult)
        tt(q[:], z_t[:], h, AL.mult)
        if not last:
            # p1m = -(a*q + gi_r) = (q * -a) - gi_r  (negated: ar is one stt)
            p1r = scanp.tile([128, CC], f16, tag="p1r")
            p1z = scanp.tile([128, CC], f16, tag="p1z")
            stt(p1r[:], q[:], na_s, gcp_n[:, 0:CC], AL.mult, AL.subtract)
            stt(p1z[:], q[:], nb_s, gcp_n[:, CC : 2 * CC], AL.mult, AL.subtract)
            # chain: ar(w+1) = p1r_true - a*u = (u * -a) - p1m
            ar = scanp.tile([128, CC], f16, tag="ar")
            az = scanp.tile([128, CC], f16, tag="az")
            stt(ar[:], u[:], na_s, p1r[:], AL.mult, AL.subtract)
            stt(az[:], u[:], nb_s, p1z[:], AL.mult, AL.subtract)
            r_t = scanp.tile([128, CC], f16, tag="r")
            z_t = scanp.tile([128, CC], f16, tag="z")
            nc.scalar.activation(r_t[:], ar[:], AF.Sigmoid)
            nc.scalar.activation(z_t[:], az[:], AF.Sigmoid)
        # h' = q - u (sigmoid shadow; single-semaphore deps for v/q next step)
        tt(h_ap(w + 1), q[:], u[:], AL.subtract)
        gcp = gcp_n

        if w - 1 in gends:
            emit_out_dma(gends[w - 1])
    emit_out_dma(NG - 1)


def _prep_core_inputs(inputs, core):
    x = inputs["inputs"]          # (W,E,B,I,F) f32
    state = inputs["state"]       # (1,E,BI,1)
    wl = inputs["weight_linear"]  # (E,16,F)
    bl = inputs["bias_linear"]    # (E,16)
    wih = inputs["weight_ih"]     # (E,3,16)
    whh = inputs["weight_hh"]     # (E,3,1)
    bih = inputs["bias_ih"]       # (E,3)
    bhh = inputs["bias_hh"]       # (E,3)

    es = slice(core * E_LOC, (core + 1) * E_LOC)
    # fold weights
    Wc = np.einsum("egp,epf->egf", wih[es], wl[es])          # (2,3,F)
    bc = np.einsum("egp,ep->eg", wih[es], bl[es]) + bih[es]  # (2,3)
    bc = bc.copy()
    bc[:, 0] += bhh[es][:, 0]
    bc[:, 1] += bhh[es][:, 1]

    # x -> (128, F, W, CC) fp16 (f-major so matmul rhs slabs are contiguous)
    xr = np.asarray(x[:, es]).reshape(W, E_LOC, PP, CC, F)
    xh = np.ascontiguousarray(xr.transpose(1, 2, 4, 0, 3)).reshape(128, F, W, CC)
    xh = xh.astype(np.float16)

    # diags (128, 27, 128) fp16
    pe = np.repeat(np.arange(E_LOC), PP)  # (128,) member index per partition
    dgv = np.zeros((128, NDIAG), np.float32)
    for g in range(G):
        for f in range(F):
            dgv[:, g * F + f] = Wc[pe, g, f]
        dgv[:, 24 + g] = bc[pe, g]
    dg = np.zeros((128, NDIAG, 128), np.float16)
    idx = np.arange(128)
    dg[idx, :, idx] = dgv.astype(np.float16)
    dg = dg.reshape(128, NDIAG * 128)

    # consts (128, 7+CC) f32
    cstv = np.zeros((128, 7 + CC), np.float32)
    cstv[:, 0] = whh[es][pe, 0, 0]
    cstv[:, 1] = whh[es][pe, 1, 0]
    cstv[:, 2] = whh[es][pe, 2, 0]
    cstv[:, 3] = bhh[es][pe, 2]
    cstv[:, 5] = -cstv[:, 0]
    cstv[:, 6] = -cstv[:, 1]
    h0 = np.asarray(state[-1, es, :, 0]).reshape(E_LOC, PP, CC)
    cstv[:, 7:] = h0.reshape(128, CC)

    return {"xh": xh, "dg": dg, "cst": cstv}


def kernel(**inputs):
    from concourse.bass_utils import run_bass_kernel_spmd

    bhh = np.asarray(inputs["bias_hh"])
    d_nonzero = bool(np.any(bhh[:, 2] != 0))

    key = ("nc", d_nonzero)
    if key not in _CACHED:
        _CACHED[key] = _build_nc(d_nonzero)
    nc = _CACHED[key]

    in_maps = [_prep_core_inputs(inputs, c) for c in range(NCORES)]
    res = run_bass_kernel_spmd(nc, in_maps, core_ids=list(range(NCORES)))

    # reassemble: per-core out (128, W*CC) f16 -> (W, E_LOC, BI)
    full = np.zeros((W, E, B, I, 1), np.float32)
    for c in range(NCORES):
        o = np.asarray(res.results[c]["out"]).astype(np.float32)
        o = o.reshape(E_LOC, PP, W, CC).transpose(2, 0, 1, 3).reshape(W, E_LOC, BI)
        full[:, c * E_LOC : (c + 1) * E_LOC] = o.reshape(W, E_LOC, B, I, 1)
    return full
